# revision 6
# baseline (speedup 1.0000x reference)
"""Trainium2 Bass kernel for nn_DomainAwareLinear — v3.

Same math as v2 (fp16 x/W, fp32 PSUM, fp16 y out, per-slice leading DMAs)
but with consolidated tile pools: one big W tile per sample and one x tile
per t-block instead of many small bufs. The Tile framework's semaphore
init/teardown cost scales with buffer count, which showed up as ~6 us of
engine-queue preamble and ~10 us of exit barrier in the v1 trace.
"""

import numpy as np

B = 16
T = 2048
I_SIZE = 2048
O_SIZE = 2048
N_CORES = 8
S = B // N_CORES  # samples per core

TRACE = False
LAST_EXEC_TIME_NS = None

_BUILD_CACHE = {}


def build_bass(s=S, t=T, i_size=I_SIZE, o_size=O_SIZE):
    key = (s, t, i_size, o_size)
    if key in _BUILD_CACHE:
        return _BUILD_CACHE[key]

    import concourse.bacc as bacc
    import concourse.bass as bass
    import concourse.mybir as mybir
    import concourse.tile as tile
    from concourse.bass import ds, ts

    P = 128
    KT = i_size // P          # contraction subtiles of 128
    TBLK = min(512, t)        # t-block held per x tile
    NT = t // TBLK
    MS = TBLK // P            # matmul lhsT tiles per t-block
    NBLK = min(512, o_size)   # o-block = PSUM free dim
    NO = o_size // NBLK

    nc = bacc.Bacc("TRN2", target_bir_lowering=False, debug=False)
    # DRAM layouts mirror the SBUF tile layouts dim-for-dim so whole-tile
    # DMAs pair src/dst bytes in the same order.
    xt_ap = nc.dram_tensor(
        "xt", [s, NT, P, MS, KT, P], mybir.dt.float16, kind="ExternalInput"
    ).ap()
    w_ap = nc.dram_tensor(
        "w", [s, P, NO, KT, NBLK], mybir.dt.float16, kind="ExternalInput"
    ).ap()
    b_ap = nc.dram_tensor(
        "bias", [s, o_size], mybir.dt.float32, kind="ExternalInput"
    ).ap()
    y_ap = nc.dram_tensor(
        "y", [s, t, o_size], mybir.dt.float16, kind="ExternalOutput"
    ).ap()

    with tile.TileContext(nc) as tc:
        with (
            tc.tile_pool(name="wpool", bufs=s) as wpool,
            tc.tile_pool(name="xpool", bufs=2) as xpool,
            tc.tile_pool(name="opool", bufs=2) as opool,
            tc.tile_pool(name="bpool", bufs=1) as bpool,
            tc.tile_pool(name="pspool", bufs=4, space="PSUM") as pspool,
        ):
            # PE warmup while the leading DMA slices land.
            warm_x = wpool.tile([P, P], mybir.dt.float16, tag="warmx", bufs=1)
            nc.vector.memset(warm_x, 0.0)
            warm_ps = pspool.tile([P, P], mybir.dt.float32, tag="warmps", bufs=1)
            for _ in range(16):
                nc.tensor.matmul(warm_ps, lhsT=warm_x, rhs=warm_x, start=True, stop=True)

            # One W tile per sample: [P, NO, KT, NBLK] (32 KB/partition each).
            # Each dma_start costs ~0.6-0.7 us of engine issue time, so the
            # startup-critical tiles are split only coarsely: w00 in four
            # 4-k quarters, x000 in two 8-k halves. W rides sync, x rides
            # scalar, gpsimd keeps only bias (+ y stores later) so no queue
            # serializes another's critical data.
            w_sb = []
            bias_sbs = []
            x_first = None
            for si in range(s):
                wt = wpool.tile([P, NO, KT, NBLK], mybir.dt.float16, tag="w")
                if si == 0:
                    for q in range(4):
                        nc.sync.dma_start(
                            out=wt[:, 0, ds(q * 4, 4), :],
                            in_=w_ap[0][:, 0, ds(q * 4, 4), :],
                        )
                    for n in range(1, NO):
                        nc.sync.dma_start(out=wt[:, n, :, :], in_=w_ap[0][:, n, :, :])
                    x_first = xpool.tile([P, MS, KT, P], mybir.dt.float16, tag="x")
                    for h in range(2):
                        nc.scalar.dma_start(
                            out=x_first[:, 0, ds(h * 8, 8), :],
                            in_=xt_ap[0][0][:, 0, ds(h * 8, 8), :],
                        )
                    for msc in range(1, MS):
                        nc.scalar.dma_start(
                            out=x_first[:, msc, :, :], in_=xt_ap[0][0][:, msc, :, :]
                        )
                else:
                    nc.sync.dma_start(out=wt, in_=w_ap[si])
                w_sb.append(wt)

                b_src = bpool.tile([1, o_size], mybir.dt.float32, tag="bsrc", bufs=1)
                nc.gpsimd.dma_start(out=b_src, in_=b_ap[si].unsqueeze(0))
                bias_sb = bpool.tile([P, o_size], mybir.dt.float32, tag="bias", bufs=s)
                nc.gpsimd.partition_broadcast(bias_sb, b_src)
                bias_sbs.append(bias_sb)

            for si in range(s):
                for tb in range(NT):
                    if si == 0 and tb == 0:
                        x_t = x_first
                    else:
                        x_t = xpool.tile([P, MS, KT, P], mybir.dt.float16, tag="x")
                        nc.scalar.dma_start(out=x_t, in_=xt_ap[si][tb])
                    for n in range(NO):
                        for ms in range(MS):
                            ps = pspool.tile([P, NBLK], mybir.dt.float32, tag="ps")
                            for k in range(KT):
                                nc.tensor.matmul(
                                    ps,
                                    lhsT=x_t[:, ms, k, :],
                                    rhs=w_sb[si][:, n, k, :],
                                    start=(k == 0),
                                    stop=(k == KT - 1),
                                )
                            o_sb = opool.tile([P, NBLK], mybir.dt.float16, tag="o")
                            nc.vector.tensor_add(
                                o_sb, ps, bias_sbs[si][:, ts(n, NBLK)]
                            )
                            nc.gpsimd.dma_start(
                                out=y_ap[si][ds(tb * TBLK + ms * P, P), ts(n, NBLK)],
                                in_=o_sb,
                            )

    nc.compile()
    _BUILD_CACHE[key] = nc
    return nc


def kernel(x, domain_id, fc_weight, bias_weight):
    global LAST_EXEC_TIME_NS
    from concourse.bass_utils import run_bass_kernel_spmd

    x = np.asarray(x)
    dom = np.asarray(domain_id).astype(np.int64)
    fc_weight = np.asarray(fc_weight)
    bias_weight = np.asarray(bias_weight)

    assert x.shape == (B, T, I_SIZE), x.shape
    assert dom.shape == (B,), dom.shape

    P, KT, NT, MS, NBLK, NO = 128, 16, 4, 4, 512, 4
    w_g = fc_weight[dom].reshape(B, KT, P, NO, NBLK).astype(np.float16)
    w_g = np.ascontiguousarray(w_g.transpose(0, 2, 3, 1, 4))
    b_g = bias_weight[dom].astype(np.float32)
    xt = x.astype(np.float16).reshape(B, NT, MS, P, KT, P)
    xt = np.ascontiguousarray(xt.transpose(0, 1, 5, 2, 4, 3))

    nc = build_bass()

    in_maps = []
    for c in range(N_CORES):
        sl = slice(c * S, (c + 1) * S)
        in_maps.append({"xt": xt[sl], "w": w_g[sl], "bias": b_g[sl]})

    kwargs = {}
    if TRACE:
        kwargs["trace"] = True
    res = run_bass_kernel_spmd(nc, in_maps, core_ids=list(range(N_CORES)), **kwargs)
    LAST_EXEC_TIME_NS = res.exec_time_ns

    y = np.concatenate([r["y"] for r in res.results], axis=0)
    return np.ascontiguousarray(y.astype(np.float32))


# revision 10
# speedup vs baseline: 1.0028x; 1.0028x over previous
"""Trainium2 Bass kernel for nn_DomainAwareLinear — v3.

Same math as v2 (fp16 x/W, fp32 PSUM, fp16 y out, per-slice leading DMAs)
but with consolidated tile pools: one big W tile per sample and one x tile
per t-block instead of many small bufs. The Tile framework's semaphore
init/teardown cost scales with buffer count, which showed up as ~6 us of
engine-queue preamble and ~10 us of exit barrier in the v1 trace.
"""

import numpy as np

B = 16
T = 2048
I_SIZE = 2048
O_SIZE = 2048
N_CORES = 8
S = B // N_CORES  # samples per core

TRACE = False
LAST_EXEC_TIME_NS = None

_BUILD_CACHE = {}


def build_bass(s=S, t=T, i_size=I_SIZE, o_size=O_SIZE):
    key = (s, t, i_size, o_size)
    if key in _BUILD_CACHE:
        return _BUILD_CACHE[key]

    import concourse.bacc as bacc
    import concourse.bass as bass
    import concourse.mybir as mybir
    import concourse.tile as tile
    from concourse.bass import ds, ts

    P = 128
    KT = i_size // P          # contraction subtiles of 128
    TBLK = min(512, t)        # t-block held per x tile
    NT = t // TBLK
    MS = TBLK // P            # matmul lhsT tiles per t-block
    NBLK = min(512, o_size)   # o-block = PSUM free dim
    NO = o_size // NBLK

    nc = bacc.Bacc("TRN2", target_bir_lowering=False, debug=False)
    # DRAM layouts mirror the SBUF tile layouts dim-for-dim so whole-tile
    # DMAs pair src/dst bytes in the same order.
    xt_ap = nc.dram_tensor(
        "xt", [s, NT, P, MS, KT, P], mybir.dt.float16, kind="ExternalInput"
    ).ap()
    w_ap = nc.dram_tensor(
        "w", [s, P, NO, KT, NBLK], mybir.dt.float16, kind="ExternalInput"
    ).ap()
    b_ap = nc.dram_tensor(
        "bias", [s, o_size], mybir.dt.float32, kind="ExternalInput"
    ).ap()
    y_ap = nc.dram_tensor(
        "y", [s, t, o_size], mybir.dt.float16, kind="ExternalOutput"
    ).ap()

    with tile.TileContext(nc) as tc:
        with (
            tc.tile_pool(name="wpool", bufs=s) as wpool,
            tc.tile_pool(name="xpool", bufs=2) as xpool,
            tc.tile_pool(name="opool", bufs=4) as opool,
            tc.tile_pool(name="bpool", bufs=1) as bpool,
            tc.tile_pool(name="pspool", bufs=4, space="PSUM") as pspool,
        ):
            # PE warmup while the leading DMA slices land.
            warm_x = wpool.tile([P, P], mybir.dt.float16, tag="warmx", bufs=1)
            nc.vector.memset(warm_x, 0.0)
            warm_ps = pspool.tile([P, P], mybir.dt.float32, tag="warmps", bufs=1)
            for _ in range(16):
                nc.tensor.matmul(warm_ps, lhsT=warm_x, rhs=warm_x, start=True, stop=True)

            # One W tile per sample: [P, NO, KT, NBLK] (32 KB/partition each).
            # Each dma_start costs ~0.6-0.7 us of engine issue time, so the
            # startup-critical tiles are split only coarsely: w00 in four
            # 4-k quarters, x000 in two 8-k halves. W rides sync, x rides
            # scalar (with the fp16 y stores), gpsimd keeps only bias so no queue
            # serializes another's critical data.
            w_sb = []
            bias_sbs = []
            x_first = None
            for si in range(s):
                wt = wpool.tile([P, NO, KT, NBLK], mybir.dt.float16, tag="w")
                if si == 0:
                    for q in range(4):
                        nc.sync.dma_start(
                            out=wt[:, 0, ds(q * 4, 4), :],
                            in_=w_ap[0][:, 0, ds(q * 4, 4), :],
                        )
                    for n in range(1, NO):
                        nc.sync.dma_start(out=wt[:, n, :, :], in_=w_ap[0][:, n, :, :])
                    # x000 quarters match the w00 quarters' k-ranges so the
                    # k-outer first block consumes both streams in lockstep.
                    x_first = xpool.tile([P, MS, KT, P], mybir.dt.float16, tag="x")
                    for q in range(4):
                        nc.scalar.dma_start(
                            out=x_first[:, :, ds(q * 4, 4), :],
                            in_=xt_ap[0][0][:, :, ds(q * 4, 4), :],
                        )
                else:
                    nc.sync.dma_start(out=wt, in_=w_ap[si])
                w_sb.append(wt)

                b_src = bpool.tile([1, o_size], mybir.dt.float32, tag="bsrc", bufs=1)
                nc.gpsimd.dma_start(out=b_src, in_=b_ap[si].unsqueeze(0))
                bias_sb = bpool.tile([P, o_size], mybir.dt.float32, tag="bias", bufs=s)
                nc.gpsimd.partition_broadcast(bias_sb, b_src)
                bias_sbs.append(bias_sb)

            for si in range(s):
                for tb in range(NT):
                    if si == 0 and tb == 0:
                        x_t = x_first
                    else:
                        x_t = xpool.tile([P, MS, KT, P], mybir.dt.float16, tag="x")
                        nc.scalar.dma_start(out=x_t, in_=xt_ap[si][tb])
                    for n in range(NO):
                        if si == 0 and tb == 0 and n == 0:
                            # k-outer across all four ms groups (4 PSUM banks
                            # accumulate concurrently): one W/x quarter feeds
                            # 16 matmuls, so the PE never outruns the DMA
                            # stream and the HAM clock-gate stays hot.
                            pss = [
                                pspool.tile(
                                    [P, NBLK],
                                    mybir.dt.float32,
                                    tag="ps",
                                    name=f"ps_first{ms}",
                                )
                                for ms in range(MS)
                            ]
                            for k in range(KT):
                                for ms in range(MS):
                                    nc.tensor.matmul(
                                        pss[ms],
                                        lhsT=x_t[:, ms, k, :],
                                        rhs=w_sb[si][:, n, k, :],
                                        start=(k == 0),
                                        stop=(k == KT - 1),
                                    )
                            for ms in range(MS):
                                o_sb = opool.tile(
                                    [P, NBLK], mybir.dt.float16, tag="o"
                                )
                                nc.vector.tensor_add(
                                    o_sb, pss[ms], bias_sbs[si][:, ts(n, NBLK)]
                                )
                                nc.scalar.dma_start(
                                    out=y_ap[si][ds(ms * P, P), ts(n, NBLK)],
                                    in_=o_sb,
                                )
                            continue
                        for ms in range(MS):
                            ps = pspool.tile([P, NBLK], mybir.dt.float32, tag="ps")
                            for k in range(KT):
                                nc.tensor.matmul(
                                    ps,
                                    lhsT=x_t[:, ms, k, :],
                                    rhs=w_sb[si][:, n, k, :],
                                    start=(k == 0),
                                    stop=(k == KT - 1),
                                )
                            o_sb = opool.tile([P, NBLK], mybir.dt.float16, tag="o")
                            nc.vector.tensor_add(
                                o_sb, ps, bias_sbs[si][:, ts(n, NBLK)]
                            )
                            nc.scalar.dma_start(
                                out=y_ap[si][ds(tb * TBLK + ms * P, P), ts(n, NBLK)],
                                in_=o_sb,
                            )

    nc.compile()
    _BUILD_CACHE[key] = nc
    return nc


def kernel(x, domain_id, fc_weight, bias_weight):
    global LAST_EXEC_TIME_NS
    from concourse.bass_utils import run_bass_kernel_spmd

    x = np.asarray(x)
    dom = np.asarray(domain_id).astype(np.int64)
    fc_weight = np.asarray(fc_weight)
    bias_weight = np.asarray(bias_weight)

    assert x.shape == (B, T, I_SIZE), x.shape
    assert dom.shape == (B,), dom.shape

    P, KT, NT, MS, NBLK, NO = 128, 16, 4, 4, 512, 4
    w_g = fc_weight[dom].reshape(B, KT, P, NO, NBLK).astype(np.float16)
    w_g = np.ascontiguousarray(w_g.transpose(0, 2, 3, 1, 4))
    b_g = bias_weight[dom].astype(np.float32)
    xt = x.astype(np.float16).reshape(B, NT, MS, P, KT, P)
    xt = np.ascontiguousarray(xt.transpose(0, 1, 5, 2, 4, 3))

    nc = build_bass()

    in_maps = []
    for c in range(N_CORES):
        sl = slice(c * S, (c + 1) * S)
        in_maps.append({"xt": xt[sl], "w": w_g[sl], "bias": b_g[sl]})

    kwargs = {}
    if TRACE:
        kwargs["trace"] = True
    res = run_bass_kernel_spmd(nc, in_maps, core_ids=list(range(N_CORES)), **kwargs)
    LAST_EXEC_TIME_NS = res.exec_time_ns

    y = np.concatenate([r["y"] for r in res.results], axis=0)
    return np.ascontiguousarray(y.astype(np.float32))


# revision 11
# speedup vs baseline: 1.0348x; 1.0319x over previous
"""Trainium2 Bass kernel for nn_DomainAwareLinear.

y[b] = x[b] @ fc_weight[domain_id[b]].reshape(I, O) + bias_weight[domain_id[b]]

Strategy: data-parallel over the batch across 8 NeuronCores (2 samples per
core). The host gathers each sample's weight row, reshapes it to [I, O],
casts x / W to fp16 (fp32 PSUM accumulation on the PE), and pre-transposes
x to x^T [I, T] so the contraction dim lands on SBUF partitions. Each core
runs dense 2048^3 matmuls per sample with the weight cached in SBUF.
"""

import numpy as np

B = 16
T = 2048
I_SIZE = 2048
O_SIZE = 2048
N_CORES = 8
S = B // N_CORES  # samples per core

# Set by test harnesses to collect HW profile timing; harmless if left False.
TRACE = False
LAST_EXEC_TIME_NS = None

_BUILD_CACHE = {}


def build_bass(s=S, t=T, i_size=I_SIZE, o_size=O_SIZE):
    """Build + compile the per-core Bass program (identical on all cores)."""
    key = (s, t, i_size, o_size)
    if key in _BUILD_CACHE:
        return _BUILD_CACHE[key]

    import concourse.bacc as bacc
    import concourse.bass as bass
    import concourse.mybir as mybir
    import concourse.tile as tile
    from concourse.bass import ds, ts

    P = 128
    KT = i_size // P          # contraction subtiles of 128
    TBLK = min(512, t)        # t-block held per x tile
    NT = t // TBLK
    MS = TBLK // P            # matmul lhsT tiles per t-block
    NBLK = min(512, o_size)   # o-block = PSUM free dim
    NO = o_size // NBLK

    nc = bacc.Bacc("TRN2", target_bir_lowering=False, debug=False)
    # x and W arrive pre-packed on the host into partition-major tile
    # layout, so every load is long-contiguous per partition. x is further
    # split into MS chunks per t-block so the first matmul group only
    # waits on 512 KB of x.
    xt_ap = nc.dram_tensor(
        "xt", [s, NT, MS, P, KT, P], mybir.dt.float16, kind="ExternalInput"
    ).ap()
    w_ap = nc.dram_tensor(
        "w", [s, NO, P, KT, NBLK], mybir.dt.float16, kind="ExternalInput"
    ).ap()
    b_ap = nc.dram_tensor(
        "bias", [s, o_size], mybir.dt.float32, kind="ExternalInput"
    ).ap()
    y_ap = nc.dram_tensor(
        "y", [s, t, o_size], mybir.dt.float16, kind="ExternalOutput"
    ).ap()

    with tile.TileContext(nc) as tc:
        with (
            tc.tile_pool(name="wpool", bufs=s * NO) as wpool,
            tc.tile_pool(name="xpool", bufs=2 * MS) as xpool,
            tc.tile_pool(name="opool", bufs=4) as opool,
            tc.tile_pool(name="bpool", bufs=s) as bpool,
            tc.tile_pool(name="pspool", bufs=6, space="PSUM") as pspool,
        ):
            # PE warmup: dummy matmuls issued during the initial DMA fill so
            # the HAM clock-gate is already at 2.4 GHz when real work starts.
            warm_x = wpool.tile([P, P], mybir.dt.float16, tag="warmx", bufs=1)
            nc.vector.memset(warm_x, 0.0)
            warm_ps = pspool.tile([P, P], mybir.dt.float32, tag="warmps", bufs=1)
            for _ in range(160):
                nc.tensor.matmul(warm_ps, lhsT=warm_x, rhs=warm_x, start=True, stop=True)

            # Hoist all weight/bias loads: W chunks on the sync HWDGE ring
            # (x and y traffic lives on the scalar ring), biases on gpsimd.
            # The o-loop below is outermost per t-block so the first matmuls
            # only wait on W chunk 0 + one 512 KB x chunk. x chunks 1-3 of
            # the very first t-block ride the sync ring BEHIND w00: the ring
            # FIFO keeps them from stealing fabric from the critical w00.
            w_sb = []
            bias_sbs = []
            x_first = None
            for si in range(s):
                chunks = []
                for n in range(NO):
                    wt = wpool.tile([P, KT, NBLK], mybir.dt.float16, tag="w")
                    nc.sync.dma_start(out=wt, in_=w_ap[si][n])
                    chunks.append(wt)
                    if si == 0 and n == 0:
                        x_first = []
                        for msc in range(MS):
                            x_c = xpool.tile([P, KT, P], mybir.dt.float16, tag="x")
                            eng = nc.scalar if msc == 0 else nc.sync
                            eng.dma_start(out=x_c, in_=xt_ap[0][0][msc])
                            x_first.append(x_c)
                w_sb.append(chunks)

                # Tiny [1, O] DMA + on-chip partition broadcast keeps the
                # bias off the HBM critical path at kernel start.
                b_src = bpool.tile([1, o_size], mybir.dt.float32, tag="bsrc", bufs=1)
                nc.gpsimd.dma_start(out=b_src, in_=b_ap[si].unsqueeze(0))
                bias_sb = bpool.tile([P, o_size], mybir.dt.float32, tag="bias")
                nc.gpsimd.partition_broadcast(bias_sb, b_src)
                bias_sbs.append(bias_sb)

            for si in range(s):
                for tb in range(NT):
                    if si == 0 and tb == 0:
                        x_cs = x_first
                    else:
                        x_cs = []
                        for msc in range(MS):
                            x_c = xpool.tile([P, KT, P], mybir.dt.float16, tag="x")
                            nc.scalar.dma_start(out=x_c, in_=xt_ap[si][tb][msc])
                            x_cs.append(x_c)
                    for n in range(NO):
                        for ms in range(MS):
                            ps = pspool.tile([P, NBLK], mybir.dt.float32, tag="ps")
                            for k in range(KT):
                                nc.tensor.matmul(
                                    ps,
                                    lhsT=x_cs[ms][:, k, :],
                                    rhs=w_sb[si][n][:, k, :],
                                    start=(k == 0),
                                    stop=(k == KT - 1),
                                )
                            o_sb = opool.tile([P, NBLK], mybir.dt.float16, tag="o")
                            nc.vector.tensor_add(
                                o_sb, ps, bias_sbs[si][:, ts(n, NBLK)]
                            )
                            nc.scalar.dma_start(
                                out=y_ap[si][ds(tb * TBLK + ms * P, P), ts(n, NBLK)],
                                in_=o_sb,
                            )

    nc.compile()
    _BUILD_CACHE[key] = nc
    return nc


def kernel(x, domain_id, fc_weight, bias_weight):
    global LAST_EXEC_TIME_NS
    from concourse.bass_utils import run_bass_kernel_spmd

    x = np.asarray(x)
    dom = np.asarray(domain_id).astype(np.int64)
    fc_weight = np.asarray(fc_weight)
    bias_weight = np.asarray(bias_weight)

    assert x.shape == (B, T, I_SIZE), x.shape
    assert dom.shape == (B,), dom.shape

    # Host-side shard prep: gather per-sample weight rows, cast to fp16,
    # and pack x / W into the partition-major tile layout the kernel loads
    # ([.., P, KT, block]: per-partition data is one long contiguous run).
    P, KT, NT, MS, NBLK, NO = 128, 16, 4, 4, 512, 4
    w_g = fc_weight[dom].reshape(B, KT, P, NO, NBLK).astype(np.float16)
    w_g = np.ascontiguousarray(w_g.transpose(0, 3, 2, 1, 4))
    b_g = bias_weight[dom].astype(np.float32)
    xt = x.astype(np.float16).reshape(B, NT, MS, P, KT, P)
    xt = np.ascontiguousarray(xt.transpose(0, 1, 2, 5, 4, 3))

    nc = build_bass()

    in_maps = []
    for c in range(N_CORES):
        sl = slice(c * S, (c + 1) * S)
        in_maps.append({"xt": xt[sl], "w": w_g[sl], "bias": b_g[sl]})

    kwargs = {}
    if TRACE:
        kwargs["trace"] = True
    res = run_bass_kernel_spmd(nc, in_maps, core_ids=list(range(N_CORES)), **kwargs)
    LAST_EXEC_TIME_NS = res.exec_time_ns

    y = np.concatenate([r["y"] for r in res.results], axis=0)
    return np.ascontiguousarray(y.astype(np.float32))



# revision 12
# speedup vs baseline: 1.0369x; 1.0020x over previous
"""Trainium2 Bass kernel for nn_DomainAwareLinear.

y[b] = x[b] @ fc_weight[domain_id[b]].reshape(I, O) + bias_weight[domain_id[b]]

Strategy: data-parallel over the batch across 8 NeuronCores (2 samples per
core). The host gathers each sample's weight row, reshapes it to [I, O],
casts x / W to fp16 (fp32 PSUM accumulation on the PE), and pre-transposes
x to x^T [I, T] so the contraction dim lands on SBUF partitions. Each core
runs dense 2048^3 matmuls per sample with the weight cached in SBUF.
"""

import numpy as np

B = 16
T = 2048
I_SIZE = 2048
O_SIZE = 2048
N_CORES = 8
S = B // N_CORES  # samples per core

# Set by test harnesses to collect HW profile timing; harmless if left False.
TRACE = False
LAST_EXEC_TIME_NS = None

_BUILD_CACHE = {}


def build_bass(s=S, t=T, i_size=I_SIZE, o_size=O_SIZE):
    """Build + compile the per-core Bass program (identical on all cores)."""
    key = (s, t, i_size, o_size)
    if key in _BUILD_CACHE:
        return _BUILD_CACHE[key]

    import concourse.bacc as bacc
    import concourse.bass as bass
    import concourse.mybir as mybir
    import concourse.tile as tile
    from concourse.bass import ds, ts

    P = 128
    KT = i_size // P          # contraction subtiles of 128
    TBLK = min(512, t)        # t-block held per x tile
    NT = t // TBLK
    MS = TBLK // P            # matmul lhsT tiles per t-block
    NBLK = min(512, o_size)   # o-block = PSUM free dim
    NO = o_size // NBLK

    nc = bacc.Bacc("TRN2", target_bir_lowering=False, debug=False)
    # x and W arrive pre-packed on the host into partition-major tile
    # layout, so every load is long-contiguous per partition. x is further
    # split into MS chunks per t-block so the first matmul group only
    # waits on 512 KB of x.
    xt_ap = nc.dram_tensor(
        "xt", [s, NT, MS, P, KT, P], mybir.dt.float16, kind="ExternalInput"
    ).ap()
    w_ap = nc.dram_tensor(
        "w", [s, NO, P, KT, NBLK], mybir.dt.float16, kind="ExternalInput"
    ).ap()
    b_ap = nc.dram_tensor(
        "bias", [s, o_size], mybir.dt.float32, kind="ExternalInput"
    ).ap()
    y_ap = nc.dram_tensor(
        "y", [s, t, o_size], mybir.dt.float16, kind="ExternalOutput"
    ).ap()

    with tile.TileContext(nc) as tc:
        with (
            tc.tile_pool(name="wpool", bufs=s * NO) as wpool,
            tc.tile_pool(name="xpool", bufs=2 * MS) as xpool,
            tc.tile_pool(name="opool", bufs=6) as opool,
            tc.tile_pool(name="bpool", bufs=s) as bpool,
            tc.tile_pool(name="pspool", bufs=7, space="PSUM") as pspool,
        ):
            # PE warmup: dummy matmuls issued during the initial DMA fill so
            # the HAM clock-gate is already at 2.4 GHz when real work starts.
            warm_x = wpool.tile([P, P], mybir.dt.float16, tag="warmx", bufs=1)
            nc.vector.memset(warm_x, 0.0)
            warm_ps = pspool.tile([P, P], mybir.dt.float32, tag="warmps", bufs=1)
            for _ in range(96):
                nc.tensor.matmul(warm_ps, lhsT=warm_x, rhs=warm_x, start=True, stop=True)

            # Hoist all weight/bias loads: W chunks on the sync HWDGE ring
            # (x and y traffic lives on the scalar ring), biases on gpsimd.
            # The o-loop below is outermost per t-block so the first matmuls
            # only wait on W chunk 0 + one 512 KB x chunk. x chunks 1-3 of
            # the very first t-block ride the sync ring BEHIND w00: the ring
            # FIFO keeps them from stealing fabric from the critical w00.
            w_sb = []
            bias_sbs = []
            x_first = None
            for si in range(s):
                chunks = []
                for n in range(NO):
                    wt = wpool.tile([P, KT, NBLK], mybir.dt.float16, tag="w")
                    if si == 0 and n == 0:
                        # First W tile: halves on parallel queues so the first
                        # matmul group's data (w00 + x000, 2.5 MB) lands at the
                        # HBM-limited ~15us instead of ~19us. Scalar carries
                        # x000 first, then w00's upper half.
                        from concourse.bass import ds as _ds
                        nc.sync.dma_start(
                            out=wt[:, _ds(0, 8), :], in_=w_ap[0][0][:, _ds(0, 8), :]
                        )
                        x_first = []
                        x_c0 = xpool.tile([P, KT, P], mybir.dt.float16, tag="x")
                        nc.scalar.dma_start(out=x_c0, in_=xt_ap[0][0][0])
                        x_first.append(x_c0)
                        nc.scalar.dma_start(
                            out=wt[:, _ds(8, 8), :], in_=w_ap[0][0][:, _ds(8, 8), :]
                        )
                        for msc in range(1, MS):
                            x_c = xpool.tile([P, KT, P], mybir.dt.float16, tag="x")
                            nc.sync.dma_start(out=x_c, in_=xt_ap[0][0][msc])
                            x_first.append(x_c)
                    else:
                        nc.sync.dma_start(out=wt, in_=w_ap[si][n])
                    chunks.append(wt)
                w_sb.append(chunks)

                # Tiny [1, O] DMA + on-chip partition broadcast keeps the
                # bias off the HBM critical path at kernel start.
                b_src = bpool.tile([1, o_size], mybir.dt.float32, tag="bsrc", bufs=1)
                nc.gpsimd.dma_start(out=b_src, in_=b_ap[si].unsqueeze(0))
                bias_sb = bpool.tile([P, o_size], mybir.dt.float32, tag="bias")
                nc.gpsimd.partition_broadcast(bias_sb, b_src)
                bias_sbs.append(bias_sb)

            for si in range(s):
                for tb in range(NT):
                    if si == 0 and tb == 0:
                        x_cs = x_first
                    else:
                        x_cs = []
                        for msc in range(MS):
                            x_c = xpool.tile([P, KT, P], mybir.dt.float16, tag="x")
                            nc.scalar.dma_start(out=x_c, in_=xt_ap[si][tb][msc])
                            x_cs.append(x_c)
                    for n in range(NO):
                        for ms in range(MS):
                            ps = pspool.tile([P, NBLK], mybir.dt.float32, tag="ps")
                            for k in range(KT):
                                nc.tensor.matmul(
                                    ps,
                                    lhsT=x_cs[ms][:, k, :],
                                    rhs=w_sb[si][n][:, k, :],
                                    start=(k == 0),
                                    stop=(k == KT - 1),
                                )
                            o_sb = opool.tile([P, NBLK], mybir.dt.float16, tag="o")
                            nc.vector.tensor_add(
                                o_sb, ps, bias_sbs[si][:, ts(n, NBLK)]
                            )
                            nc.scalar.dma_start(
                                out=y_ap[si][ds(tb * TBLK + ms * P, P), ts(n, NBLK)],
                                in_=o_sb,
                            )

    nc.compile()
    _BUILD_CACHE[key] = nc
    return nc


def kernel(x, domain_id, fc_weight, bias_weight):
    global LAST_EXEC_TIME_NS
    from concourse.bass_utils import run_bass_kernel_spmd

    x = np.asarray(x)
    dom = np.asarray(domain_id).astype(np.int64)
    fc_weight = np.asarray(fc_weight)
    bias_weight = np.asarray(bias_weight)

    assert x.shape == (B, T, I_SIZE), x.shape
    assert dom.shape == (B,), dom.shape

    # Host-side shard prep: gather per-sample weight rows, cast to fp16,
    # and pack x / W into the partition-major tile layout the kernel loads
    # ([.., P, KT, block]: per-partition data is one long contiguous run).
    P, KT, NT, MS, NBLK, NO = 128, 16, 4, 4, 512, 4
    w_g = fc_weight[dom].reshape(B, KT, P, NO, NBLK).astype(np.float16)
    w_g = np.ascontiguousarray(w_g.transpose(0, 3, 2, 1, 4))
    b_g = bias_weight[dom].astype(np.float32)
    xt = x.astype(np.float16).reshape(B, NT, MS, P, KT, P)
    xt = np.ascontiguousarray(xt.transpose(0, 1, 2, 5, 4, 3))

    nc = build_bass()

    in_maps = []
    for c in range(N_CORES):
        sl = slice(c * S, (c + 1) * S)
        in_maps.append({"xt": xt[sl], "w": w_g[sl], "bias": b_g[sl]})

    kwargs = {}
    if TRACE:
        kwargs["trace"] = True
    res = run_bass_kernel_spmd(nc, in_maps, core_ids=list(range(N_CORES)), **kwargs)
    LAST_EXEC_TIME_NS = res.exec_time_ns

    y = np.concatenate([r["y"] for r in res.results], axis=0)
    return np.ascontiguousarray(y.astype(np.float32))



# revision 13
# speedup vs baseline: 1.0428x; 1.0057x over previous
"""Trainium2 Bass kernel for nn_DomainAwareLinear.

y[b] = x[b] @ fc_weight[domain_id[b]].reshape(I, O) + bias_weight[domain_id[b]]

Strategy: data-parallel over the batch across 8 NeuronCores (2 samples per
core). The host gathers each sample's weight row, reshapes it to [I, O],
casts x / W to fp16 (fp32 PSUM accumulation on the PE), and pre-transposes
x to x^T [I, T] so the contraction dim lands on SBUF partitions. Each core
runs dense 2048^3 matmuls per sample with the weight cached in SBUF.
"""

import numpy as np

B = 16
T = 2048
I_SIZE = 2048
O_SIZE = 2048
N_CORES = 8
S = B // N_CORES  # samples per core

# Set by test harnesses to collect HW profile timing; harmless if left False.
TRACE = False
LAST_EXEC_TIME_NS = None

_BUILD_CACHE = {}


def build_bass(s=S, t=T, i_size=I_SIZE, o_size=O_SIZE):
    """Build + compile the per-core Bass program (identical on all cores)."""
    key = (s, t, i_size, o_size)
    if key in _BUILD_CACHE:
        return _BUILD_CACHE[key]

    import concourse.bacc as bacc
    import concourse.bass as bass
    import concourse.mybir as mybir
    import concourse.tile as tile
    from concourse.bass import ds, ts

    P = 128
    KT = i_size // P          # contraction subtiles of 128
    TBLK = min(512, t)        # t-block held per x tile
    NT = t // TBLK
    MS = TBLK // P            # matmul lhsT tiles per t-block
    NBLK = min(512, o_size)   # o-block = PSUM free dim
    NO = o_size // NBLK

    nc = bacc.Bacc("TRN2", target_bir_lowering=False, debug=False)
    # x and W arrive pre-packed on the host into partition-major tile
    # layout, so every load is long-contiguous per partition. x is further
    # split into MS chunks per t-block so the first matmul group only
    # waits on 512 KB of x.
    xt_ap = nc.dram_tensor(
        "xt", [s, NT, MS, P, KT, P], mybir.dt.float16, kind="ExternalInput"
    ).ap()
    w_ap = nc.dram_tensor(
        "w", [s, NO, P, KT, NBLK], mybir.dt.float16, kind="ExternalInput"
    ).ap()
    b_ap = nc.dram_tensor(
        "bias", [s, o_size], mybir.dt.float32, kind="ExternalInput"
    ).ap()
    y_ap = nc.dram_tensor(
        "y", [s, t, o_size], mybir.dt.float16, kind="ExternalOutput"
    ).ap()

    with tile.TileContext(nc) as tc:
        with (
            tc.tile_pool(name="wpool", bufs=s * NO) as wpool,
            tc.tile_pool(name="xpool", bufs=2 * MS) as xpool,
            tc.tile_pool(name="opool", bufs=6) as opool,
            tc.tile_pool(name="bpool", bufs=s) as bpool,
            tc.tile_pool(name="pspool", bufs=7, space="PSUM") as pspool,
        ):
            # PE warmup: dummy matmuls issued during the initial DMA fill so
            # the HAM clock-gate is already at 2.4 GHz when real work starts.
            warm_x = wpool.tile([P, P], mybir.dt.float16, tag="warmx", bufs=1)
            nc.vector.memset(warm_x, 0.0)
            warm_ps = pspool.tile([P, P], mybir.dt.float32, tag="warmps", bufs=1)
            for _ in range(80):
                nc.tensor.matmul(warm_ps, lhsT=warm_x, rhs=warm_x, start=True, stop=True)

            # Hoist all weight/bias loads: W chunks on the sync HWDGE ring
            # (x and y traffic lives on the scalar ring), biases on gpsimd.
            # The o-loop below is outermost per t-block so the first matmuls
            # only wait on W chunk 0 + one 512 KB x chunk. x chunks 1-3 of
            # the very first t-block ride the sync ring BEHIND w00: the ring
            # FIFO keeps them from stealing fabric from the critical w00.
            w_sb = []
            bias_sbs = []
            x_first = None
            for si in range(s):
                chunks = []
                for n in range(NO):
                    wt = wpool.tile([P, KT, NBLK], mybir.dt.float16, tag="w")
                    if si == 0 and n == 0:
                        # The whole startup-critical chain rides the sync queue
                        # (fast ramp: ~0.33 MB/us from ~9.5us; scalar's queue
                        # ramps ~2us later and slower) in exact consumption
                        # order: x000, w00 lower/upper half, x ms1-3. Matmuls
                        # then start at ~15us with no stall anywhere.
                        from concourse.bass import ds as _ds
                        x_first = []
                        x_c0 = xpool.tile([P, KT, P], mybir.dt.float16, tag="x")
                        nc.sync.dma_start(out=x_c0, in_=xt_ap[0][0][0])
                        x_first.append(x_c0)
                        nc.sync.dma_start(
                            out=wt[:, _ds(0, 8), :], in_=w_ap[0][0][:, _ds(0, 8), :]
                        )
                        nc.sync.dma_start(
                            out=wt[:, _ds(8, 8), :], in_=w_ap[0][0][:, _ds(8, 8), :]
                        )
                        for msc in range(1, MS):
                            x_c = xpool.tile([P, KT, P], mybir.dt.float16, tag="x")
                            nc.sync.dma_start(out=x_c, in_=xt_ap[0][0][msc])
                            x_first.append(x_c)
                    else:
                        nc.sync.dma_start(out=wt, in_=w_ap[si][n])
                    chunks.append(wt)
                w_sb.append(chunks)

                # Tiny [1, O] DMA + on-chip partition broadcast keeps the
                # bias off the HBM critical path at kernel start.
                b_src = bpool.tile([1, o_size], mybir.dt.float32, tag="bsrc", bufs=1)
                nc.gpsimd.dma_start(out=b_src, in_=b_ap[si].unsqueeze(0))
                bias_sb = bpool.tile([P, o_size], mybir.dt.float32, tag="bias")
                nc.gpsimd.partition_broadcast(bias_sb, b_src)
                bias_sbs.append(bias_sb)

            for si in range(s):
                for tb in range(NT):
                    if si == 0 and tb == 0:
                        x_cs = x_first
                    else:
                        x_cs = []
                        for msc in range(MS):
                            x_c = xpool.tile([P, KT, P], mybir.dt.float16, tag="x")
                            nc.scalar.dma_start(out=x_c, in_=xt_ap[si][tb][msc])
                            x_cs.append(x_c)
                    for n in range(NO):
                        for ms in range(MS):
                            ps = pspool.tile([P, NBLK], mybir.dt.float32, tag="ps")
                            for k in range(KT):
                                nc.tensor.matmul(
                                    ps,
                                    lhsT=x_cs[ms][:, k, :],
                                    rhs=w_sb[si][n][:, k, :],
                                    start=(k == 0),
                                    stop=(k == KT - 1),
                                )
                            o_sb = opool.tile([P, NBLK], mybir.dt.float16, tag="o")
                            nc.vector.tensor_add(
                                o_sb, ps, bias_sbs[si][:, ts(n, NBLK)]
                            )
                            nc.scalar.dma_start(
                                out=y_ap[si][ds(tb * TBLK + ms * P, P), ts(n, NBLK)],
                                in_=o_sb,
                            )

    nc.compile()
    _BUILD_CACHE[key] = nc
    return nc


def kernel(x, domain_id, fc_weight, bias_weight):
    global LAST_EXEC_TIME_NS
    from concourse.bass_utils import run_bass_kernel_spmd

    x = np.asarray(x)
    dom = np.asarray(domain_id).astype(np.int64)
    fc_weight = np.asarray(fc_weight)
    bias_weight = np.asarray(bias_weight)

    assert x.shape == (B, T, I_SIZE), x.shape
    assert dom.shape == (B,), dom.shape

    # Host-side shard prep: gather per-sample weight rows, cast to fp16,
    # and pack x / W into the partition-major tile layout the kernel loads
    # ([.., P, KT, block]: per-partition data is one long contiguous run).
    P, KT, NT, MS, NBLK, NO = 128, 16, 4, 4, 512, 4
    w_g = fc_weight[dom].reshape(B, KT, P, NO, NBLK).astype(np.float16)
    w_g = np.ascontiguousarray(w_g.transpose(0, 3, 2, 1, 4))
    b_g = bias_weight[dom].astype(np.float32)
    xt = x.astype(np.float16).reshape(B, NT, MS, P, KT, P)
    xt = np.ascontiguousarray(xt.transpose(0, 1, 2, 5, 4, 3))

    nc = build_bass()

    in_maps = []
    for c in range(N_CORES):
        sl = slice(c * S, (c + 1) * S)
        in_maps.append({"xt": xt[sl], "w": w_g[sl], "bias": b_g[sl]})

    kwargs = {}
    if TRACE:
        kwargs["trace"] = True
    res = run_bass_kernel_spmd(nc, in_maps, core_ids=list(range(N_CORES)), **kwargs)
    LAST_EXEC_TIME_NS = res.exec_time_ns

    y = np.concatenate([r["y"] for r in res.results], axis=0)
    return np.ascontiguousarray(y.astype(np.float32))

